# revision 6
# baseline (speedup 1.0000x reference)
"""2-layer tanh-RNN encoder on Trainium2 — replicated sequential-scan design.

Every core runs the identical program: the full layer-0 recurrence (embedding
gather fused in), then the full layer-1 recurrence consuming the transposed
layer-0 outputs staged in DRAM.  Matmuls run in fp16 (stationary x^T / h^T,
moving W) with fp32 PSUM accumulation; dual column-group tile positions
(0,0)/(0,64) compute both 512-wide n-slices of each step concurrently, so the
hidden state lands in a folded [128 = batch x 2-half, 512] layout.  The
per-step h -> h^T transposes ride the DMA XBAR (2-byte transpose), keeping
the PE free for matmuls.  Bias enters as one extra K=128 constant chunk
(ones-row stationary x bias-row moving).
"""

import numpy as np

S, B, E, H, V = 512, 64, 512, 1024, 32000
N_CORES = 8

_cache = {}


def _build(s_steps=S):
    import concourse.bass as bass
    import concourse.bacc as bacc
    import concourse.tile as tile
    from concourse import mybir

    f32 = mybir.dt.float32
    fp16 = mybir.dt.float16
    i32 = mybir.dt.int32

    nc = bacc.Bacc("TRN2", target_bir_lowering=False, debug=False,
                   num_devices=N_CORES)
    tok_d = nc.dram_tensor("tokens_t", [B, s_steps], i32, kind="ExternalInput").ap()
    emb_d = nc.dram_tensor("emb", [V, E], f32, kind="ExternalInput").ap()
    w_d = {}
    for nm, kc in [("wih0", 4), ("whh0", 8), ("wih1", 8), ("whh1", 8)]:
        w_d[nm] = nc.dram_tensor(nm, [kc, 128, H], f32, kind="ExternalInput").ap()
    b0_d = nc.dram_tensor("bias0", [1, H], f32, kind="ExternalInput").ap()
    b1_d = nc.dram_tensor("bias1", [1, H], f32, kind="ExternalInput").ap()
    y1_d = nc.dram_tensor("y1", [s_steps, B, H], f32, kind="ExternalOutput").ap()
    hid_d = nc.dram_tensor("hidden", [2, B, H], f32, kind="ExternalOutput").ap()

    with tile.TileContext(nc) as tc:
        with (
            tc.tile_pool(name="wp", bufs=1) as wp,
            tc.tile_pool(name="sb", bufs=4) as sb,
            tc.tile_pool(name="hp", bufs=3) as hp,
            tc.tile_pool(name="ps", bufs=4, space="PSUM") as ps,
            tc.tile_pool(name="dy", bufs=1, space="DRAM") as dy,
        ):
            wt = {}
            for nm, kc in [("wih0", 4), ("whh0", 8), ("wih1", 8), ("whh1", 8)]:
                wt[nm] = [wp.tile([128, H], fp16, tag=f"{nm}_{k}", name=f"{nm}_{k}")
                          for k in range(kc)]
                for k in range(kc):
                    nc.gpsimd.dma_start(wt[nm][k][:], w_d[nm][k])
            brow = {}
            for nm, bd in [("b0", b0_d), ("b1", b1_d)]:
                scr = wp.tile([128, H], f32, tag=f"scr_{nm}", name=f"scr_{nm}")
                nc.gpsimd.memset(scr[:], 0.0)
                nc.sync.dma_start(scr[0:1, :], bd)
                bt = wp.tile([128, H], fp16, tag=f"brow_{nm}", name=f"brow_{nm}")
                nc.vector.tensor_copy(bt[:], scr[:])
                brow[nm] = bt
            scr2 = wp.tile([128, 64], f32, tag="scr2")
            nc.gpsimd.memset(scr2[:], 0.0)
            nc.gpsimd.memset(scr2[0:1, :], 1.0)
            ones = wp.tile([128, 64], fp16, tag="ones")
            nc.vector.tensor_copy(ones[:], scr2[:])
            tok_t = wp.tile([B, s_steps], i32, tag="tok")
            nc.sync.dma_start(tok_t[:], tok_d)

            y0T_dram = dy.tile([s_steps, 8, 128, 64], fp16, name="y0T_dram")

            def mm_accum(z_ps, chunks):
                n = len(chunks)
                for i, (lt, rt) in enumerate(chunks):
                    for half in range(2):
                        nc.tensor.matmul(
                            z_ps[64 * half:64 * half + 64, :],
                            lt, rt[:, 512 * half:512 * half + 512],
                            start=(i == 0), stop=(i == n - 1),
                            tile_position=(0, 64 * half),
                            skip_group_check=True)

            def step_common(t, z_ps, name):
                h_full = sb.tile([128, 512], f32, tag="hfull", name=f"hf_{name}_{t}")
                nc.scalar.activation(h_full[:], z_ps[:],
                                     mybir.ActivationFunctionType.Tanh)
                h16 = sb.tile([128, 512], fp16, tag="h16", name=f"h16_{name}_{t}")
                nc.vector.tensor_copy(h16[:], h_full[:])
                hT = hp.tile([128, 512], fp16, tag=f"hT{name}", name=f"hT_{name}_{t}")
                for c in range(8):
                    half, cc = c // 4, c % 4
                    nc.sync.dma_start(
                        hT[:, c * 64:(c + 1) * 64],
                        h16[64 * half:64 * half + 64, cc * 128:(cc + 1) * 128],
                        transpose=True)
                return h_full, hT

            # ---------------- layer-0 scan ----------------
            hT_prev = None
            for t in range(s_steps):
                x_t = sb.tile([B, E], f32, tag="x", name=f"x{t}")
                nc.gpsimd.indirect_dma_start(
                    out=x_t[:], out_offset=None, in_=emb_d,
                    in_offset=bass.IndirectOffsetOnAxis(ap=tok_t[:, t:t + 1], axis=0))
                x16 = sb.tile([B, E], fp16, tag="x16", name=f"x16_{t}")
                nc.vector.tensor_copy(x16[:], x_t[:])
                xT = sb.tile([128, 256], fp16, tag="xT", name=f"xT{t}")
                for c in range(4):
                    nc.sync.dma_start(xT[:, c * 64:(c + 1) * 64],
                                      x16[:, c * 128:(c + 1) * 128], transpose=True)
                z_ps = ps.tile([128, 512], f32, tag="z0", name=f"z0_{t}")
                chunks = [(xT[:, c * 64:(c + 1) * 64], wt["wih0"][c]) for c in range(4)]
                if t > 0:
                    chunks += [(hT_prev[:, c * 64:(c + 1) * 64], wt["whh0"][c])
                               for c in range(8)]
                chunks += [(ones[:, :], brow["b0"])]
                mm_accum(z_ps, chunks)
                h_full, hT = step_common(t, z_ps, "l0")
                hT_prev = hT
                nc.sync.dma_start(y0T_dram[t].rearrange("c p m -> p c m"), hT[:])
                if t == s_steps - 1:
                    nc.sync.dma_start(
                        hid_d[0].rearrange("b (nh n) -> nh b n", nh=2), h_full[:])

            # ---------------- layer-1 scan ----------------
            hT_prev = None
            for t in range(s_steps):
                y0T = sb.tile([128, 512], fp16, tag="y0T", name=f"y0T{t}")
                nc.sync.dma_start(y0T[:], y0T_dram[t].rearrange("c p m -> p c m"))
                z_ps = ps.tile([128, 512], f32, tag="z1", name=f"z1_{t}")
                chunks = [(y0T[:, c * 64:(c + 1) * 64], wt["wih1"][c]) for c in range(8)]
                if t > 0:
                    chunks += [(hT_prev[:, c * 64:(c + 1) * 64], wt["whh1"][c])
                               for c in range(8)]
                chunks += [(ones[:, :], brow["b1"])]
                mm_accum(z_ps, chunks)
                h_full, hT = step_common(t, z_ps, "l1")
                hT_prev = hT
                nc.sync.dma_start(
                    y1_d[t].rearrange("b (nh n) -> nh b n", nh=2), h_full[:])
                if t == s_steps - 1:
                    nc.sync.dma_start(
                        hid_d[1].rearrange("b (nh n) -> nh b n", nh=2), h_full[:])

    nc.finalize()
    return nc


def kernel(tokens, emb, W_ih0, W_hh0, b_ih0, b_hh0, W_ih1, W_hh1, b_ih1, b_hh1):
    import concourse.bass_utils as bass_utils

    tokens = np.asarray(tokens)
    emb = np.ascontiguousarray(np.asarray(emb, np.float32))
    s_steps = tokens.shape[0]
    if s_steps not in _cache:
        _cache[s_steps] = _build(s_steps)
    nc = _cache[s_steps]

    def wchunks(w):
        w = np.asarray(w, np.float32)
        return np.ascontiguousarray(w.reshape(-1, 128, H))

    m = {
        "tokens_t": np.ascontiguousarray(tokens.astype(np.int32).T),
        "emb": emb,
        "wih0": wchunks(W_ih0), "whh0": wchunks(W_hh0),
        "wih1": wchunks(W_ih1), "whh1": wchunks(W_hh1),
        "bias0": (np.asarray(b_ih0) + np.asarray(b_hh0)).reshape(1, H).astype(np.float32),
        "bias1": (np.asarray(b_ih1) + np.asarray(b_hh1)).reshape(1, H).astype(np.float32),
    }
    res = bass_utils.run_bass_kernel_spmd(nc, [m] * N_CORES,
                                          core_ids=list(range(N_CORES)))
    y1 = res.results[0]["y1"]
    hidden = res.results[0]["hidden"]
    return y1, hidden
